# revision 1
# baseline (speedup 1.0000x reference)
"""Trainium2 Bass kernel for an attention block (B=16, C=512, T=2048).

reference:
  q = wq@x + bq; k = wk@x + bk; v = wv@x + bv          (conv1x1 per sample)
  attn = softmax(q^T k over s); out = v @ attn^T
  result = gamma * out + x

Sharding: data-parallel over batch across 8 NeuronCores (2 samples/core),
weights replicated.

Device algorithm (per sample), all matmuls in float32r (fp32 with 11-bit
mantissa -> full PE rate at N>=256) accumulating in fp32 PSUM:
  - gamma is folded into wv/bv on the host; bk is dropped (a per-t constant
    shift in scores cancels in softmax over s).
  - v^T[s,o] tiles via matmul(lhsT=x[c,s], rhs=(gamma*wv)^T[c,o])
  - q[d,t], k[d,s] via matmul(lhsT=wq^T/wk^T, rhs=x); bias only on q
  - per 512-wide t-chunk: for each 128-wide s-chunk:
      S^T[s,t] = matmul(lhsT=k[:,s], rhs=q[:,t])      (K=64, N=512)
      E = exp(S^T)  (ACT, PSUM->SBUF f32r; no max-subtraction: |S|<~60)
      den += matmul(lhsT=ones128, rhs=E)              (sum over s, result
                                                       broadcast on all parts)
      out0[c,t] += matmul(lhsT=v^T[s,c], rhs=E)       (4 c-chunks)
    then: result = out0 * recip(den) + gamma*bv + x   (DVE/ACT) -> DMA out
"""
import numpy as np
import concourse.bass as bass
import concourse.bacc as bacc
import concourse.tile as tile
from concourse import mybir
from concourse.bass_utils import run_bass_kernel_spmd

F32 = mybir.dt.float32
F32R = mybir.dt.float32r
AF = mybir.ActivationFunctionType

B, C, T, D = 16, 512, 2048, 64
NCORES = 8
BPC = B // NCORES          # samples per core
CCH = C // 128             # 4 channel chunks
TW = 512                   # t tile width (matmul free dim)
TCH = T // TW              # 4 t chunks
SCH = T // 128             # 16 s chunks

PROFILE = False            # set True before calling kernel() to capture HW time
LAST_EXEC_NS = None
_CACHE = {}


def _round_fp32r(a: np.ndarray) -> np.ndarray:
    """Round fp32 to fp32r precision (11 explicit mantissa bits, RNE)."""
    u = np.ascontiguousarray(a, dtype=np.float32).view(np.uint32)
    lsb = (u >> 12) & 1
    rounded = u + np.uint32(0x7FF) + lsb
    return (rounded & np.uint32(0xFFFFF000)).astype(np.uint32).view(np.float32)


def _build():
    nc = bacc.Bacc("TRN2", target_bir_lowering=False, debug=False,
                   enable_asserts=False)
    xd = nc.dram_tensor("x", [BPC, C, T], F32R, kind="ExternalInput").ap()
    wqT = nc.dram_tensor("wqT", [C, D], F32R, kind="ExternalInput").ap()
    wkT = nc.dram_tensor("wkT", [C, D], F32R, kind="ExternalInput").ap()
    wvT = nc.dram_tensor("wvT", [C, C], F32R, kind="ExternalInput").ap()
    bqd = nc.dram_tensor("bq", [D, 1], F32, kind="ExternalInput").ap()
    gbvd = nc.dram_tensor("gbv", [C, 1], F32, kind="ExternalInput").ap()
    onesd = nc.dram_tensor("ones", [128, 128], F32R, kind="ExternalInput").ap()
    outd = nc.dram_tensor("out", [BPC, C, T], F32, kind="ExternalOutput").ap()

    with tile.TileContext(nc) as tc:
        with tc.tile_pool(name="const", bufs=1) as constp, \
             tc.tile_pool(name="xp", bufs=2) as xp, \
             tc.tile_pool(name="vtp", bufs=1) as vtp, \
             tc.tile_pool(name="qkp", bufs=1) as qkp, \
             tc.tile_pool(name="etp", bufs=1) as etp, \
             tc.tile_pool(name="finp", bufs=1) as finp, \
             tc.tile_pool(name="ps", bufs=1, space="PSUM") as ps:

            # ---- constants (loaded once) ----
            wv_sb = []
            wq_sb = []
            wk_sb = []
            gbv_sb = []
            for cc in range(CCH):
                t_wv = constp.tile([128, C], F32R, name=f"wv{cc}")
                nc.sync.dma_start(out=t_wv, in_=wvT[cc * 128:(cc + 1) * 128, :])
                wv_sb.append(t_wv)
                t_wq = constp.tile([128, D], F32R, name=f"wq{cc}")
                nc.sync.dma_start(out=t_wq, in_=wqT[cc * 128:(cc + 1) * 128, :])
                wq_sb.append(t_wq)
                t_wk = constp.tile([128, D], F32R, name=f"wk{cc}")
                nc.sync.dma_start(out=t_wk, in_=wkT[cc * 128:(cc + 1) * 128, :])
                wk_sb.append(t_wk)
                t_gbv = constp.tile([128, 1], F32, name=f"gbv{cc}")
                nc.sync.dma_start(out=t_gbv, in_=gbvd[cc * 128:(cc + 1) * 128, :])
                gbv_sb.append(t_gbv)
            ones = constp.tile([128, 128], F32R)
            nc.sync.dma_start(out=ones, in_=onesd)
            bq_sb = constp.tile([D, 1], F32)
            nc.sync.dma_start(out=bq_sb, in_=bqd)

            for b in range(BPC):
                # ---- load x ----
                x_sb = []
                for cc in range(CCH):
                    t_x = xp.tile([128, T], F32R, name=f"x_{b}_{cc}",
                                  tag=f"x{cc}")
                    nc.sync.dma_start(out=t_x,
                                      in_=xd[b, cc * 128:(cc + 1) * 128, :])
                    x_sb.append(t_x)

                # ---- v^T tiles: vt[sc][s=128, o=512] ----
                vt_sb = []
                for sc in range(SCH):
                    vps = ps.tile([128, TW], F32, name=f"vps_{b}_{sc}",
                                  tag="stq", bufs=2)
                    for cc in range(CCH):
                        nc.tensor.matmul(
                            vps[:],
                            x_sb[cc][:, sc * 128:(sc + 1) * 128],
                            wv_sb[cc][:],
                            start=(cc == 0), stop=(cc == CCH - 1))
                    t_vt = vtp.tile([128, C], F32R, name=f"vt_{b}_{sc}",
                                    tag=f"vt{sc}")
                    nc.scalar.activation(out=t_vt[:], in_=vps[:], func=AF.Copy)
                    vt_sb.append(t_vt)

                # ---- q, k: [64, T] ----
                q_sb = qkp.tile([D, T], F32R, name=f"q_{b}", tag="q")
                k_sb = qkp.tile([D, T], F32R, name=f"k_{b}", tag="k")
                for tc_i in range(TCH):
                    tsl = slice(tc_i * TW, (tc_i + 1) * TW)
                    qps = ps.tile([D, TW], F32, name=f"qps_{b}_{tc_i}",
                                  tag="stq", bufs=2)
                    for cc in range(CCH):
                        nc.tensor.matmul(qps[:], wq_sb[cc][:],
                                         x_sb[cc][:, tsl],
                                         start=(cc == 0), stop=(cc == CCH - 1))
                    nc.scalar.activation(out=q_sb[:, tsl], in_=qps[:],
                                         func=AF.Identity, bias=bq_sb[:],
                                         scale=1.0)
                    kps = ps.tile([D, TW], F32, name=f"kps_{b}_{tc_i}",
                                  tag="stq", bufs=2)
                    for cc in range(CCH):
                        nc.tensor.matmul(kps[:], wk_sb[cc][:],
                                         x_sb[cc][:, tsl],
                                         start=(cc == 0), stop=(cc == CCH - 1))
                    nc.scalar.activation(out=k_sb[:, tsl], in_=kps[:],
                                         func=AF.Copy)

                # ---- attention, one 512-wide t-chunk at a time ----
                for tc_i in range(TCH):
                    tsl = slice(tc_i * TW, (tc_i + 1) * TW)
                    den = ps.tile([128, TW], F32, name=f"den_{b}_{tc_i}",
                                  tag="den")
                    oacc = []
                    for cc in range(CCH):
                        t_o = ps.tile([128, TW], F32,
                                      name=f"o_{b}_{tc_i}_{cc}", tag=f"o{cc}")
                        oacc.append(t_o)
                    for sc in range(SCH):
                        stp = ps.tile([128, TW], F32,
                                      name=f"st_{b}_{tc_i}_{sc}", tag="stq",
                                      bufs=2)
                        nc.tensor.matmul(
                            stp[:], k_sb[:, sc * 128:(sc + 1) * 128],
                            q_sb[:, tsl], start=True, stop=True)
                        et = etp.tile([128, TW], F32R,
                                      name=f"et_{b}_{tc_i}_{sc}",
                                      tag=f"et{sc}")
                        nc.scalar.activation(out=et[:], in_=stp[:],
                                             func=AF.Exp)
                        nc.tensor.matmul(den[:], ones[:], et[:],
                                         start=(sc == 0), stop=(sc == SCH - 1))
                        for cc in range(CCH):
                            nc.tensor.matmul(
                                oacc[cc][:],
                                vt_sb[sc][:, cc * 128:(cc + 1) * 128],
                                et[:], start=(sc == 0), stop=(sc == SCH - 1))

                    recip = finp.tile([128, TW], F32,
                                      name=f"rc_{b}_{tc_i}", tag="rc", bufs=2)
                    nc.vector.reciprocal(out=recip[:], in_=den[:])
                    for cc in range(CCH):
                        t_m = finp.tile([128, TW], F32,
                                        name=f"m_{b}_{tc_i}_{cc}", tag="m",
                                        bufs=2)
                        nc.vector.tensor_mul(t_m[:], oacc[cc][:], recip[:])
                        t_f = finp.tile([128, TW], F32,
                                        name=f"f_{b}_{tc_i}_{cc}", tag="f",
                                        bufs=3)
                        nc.scalar.activation(out=t_f[:], in_=t_m[:],
                                             func=AF.Identity,
                                             bias=gbv_sb[cc][:], scale=1.0)
                        nc.vector.tensor_add(t_f[:], t_f[:],
                                             x_sb[cc][:, tsl].bitcast(F32))
                        nc.sync.dma_start(
                            out=outd[b, cc * 128:(cc + 1) * 128, tsl],
                            in_=t_f)
    nc.compile()
    return nc


def _get_nc():
    if "nc" not in _CACHE:
        _CACHE["nc"] = _build()
    return _CACHE["nc"]


def kernel(x, wq, bq, wk, bk, wv, bv, gamma):
    global LAST_EXEC_NS
    g = float(np.asarray(gamma).reshape(-1)[0])
    # fold gamma into the v path; bk cancels inside softmax
    wvT = _round_fp32r((g * np.asarray(wv, np.float32)).T)
    gbv = (g * np.asarray(bv, np.float32)).reshape(C, 1)
    wqT = _round_fp32r(np.asarray(wq, np.float32).T)
    wkT = _round_fp32r(np.asarray(wk, np.float32).T)
    bq2 = np.asarray(bq, np.float32).reshape(D, 1)
    ones = np.ones((128, 128), np.float32)
    xr = _round_fp32r(np.asarray(x, np.float32))

    in_maps = []
    for core in range(NCORES):
        in_maps.append({
            "x": xr[core * BPC:(core + 1) * BPC],
            "wqT": wqT, "wkT": wkT, "wvT": wvT,
            "bq": bq2, "gbv": gbv, "ones": ones,
        })

    nc = _get_nc()
    res = run_bass_kernel_spmd(nc, in_maps, core_ids=list(range(NCORES)),
                               trace=PROFILE)
    LAST_EXEC_NS = res.exec_time_ns
    out = np.empty((B, C, T), np.float32)
    for core in range(NCORES):
        out[core * BPC:(core + 1) * BPC] = res.results[core]["out"]
    return out


# revision 2
# speedup vs baseline: 1.0471x; 1.0471x over previous
"""Trainium2 Bass kernel for an attention block (B=16, C=512, T=2048).

reference:
  q = wq@x + bq; k = wk@x + bk; v = wv@x + bv          (conv1x1 per sample)
  attn = softmax(q^T k over s); out = v @ attn^T
  result = gamma * out + x

Sharding: data-parallel over batch across 8 NeuronCores (2 samples/core),
weights replicated.

Device algorithm (per sample):
  - gamma folded into wv/bv on host; bk dropped (per-t constant in scores
    cancels in softmax over s).
  - scores path in float32r (fp32 w/ 11-bit mantissa), v/softmax-weights
    path in bf16; PSUM accumulation always fp32.
  - v^T[s,o] tiles via matmul(lhsT=x[c,s], rhs=(gamma*wv)^T[c,o]) -> bf16
  - q[d,t], k[d,s] via matmul(lhsT=wq^T/wk^T, rhs=x); bias only on q
  - per 512-wide t-chunk, for each 128-wide s-chunk (sw pipelined by 2):
      S^T[s,t] = matmul(lhsT=k[:,s], rhs=q[:,t])      (f32r, K=64, N=512)
      E = exp(S^T)  (ACT, PSUM->SBUF bf16; no max-subtraction: |S|<~64)
      den += matmul(lhsT=ones128, rhs=E)              (bf16; sum over s,
                                                       broadcast on parts)
      out0[c,t] += matmul(lhsT=v^T[s,c], rhs=E)       (bf16; 4 c-chunks)
    then: out0 -> SBUF (ACT, frees PSUM banks), r = recip(den) (DVE),
          result = out0 * r + gamma*bv + x -> DMA out
"""
import numpy as np
import concourse.bass as bass
import concourse.bacc as bacc
import concourse.tile as tile
from concourse import mybir
from concourse.bass_utils import run_bass_kernel_spmd

F32 = mybir.dt.float32
F32R = mybir.dt.float32r
BF16 = mybir.dt.bfloat16
AF = mybir.ActivationFunctionType

B, C, T, D = 16, 512, 2048, 64
NCORES = 8
BPC = B // NCORES          # samples per core
CCH = C // 128             # 4 channel chunks
TW = 512                   # t tile width (matmul free dim)
TCH = T // TW              # 4 t chunks
SCH = T // 128             # 16 s chunks

PROFILE = False            # set True before calling kernel() to capture HW time
LAST_EXEC_NS = None
_CACHE = {}


def _round_fp32r(a: np.ndarray) -> np.ndarray:
    """Round fp32 to fp32r precision (11 explicit mantissa bits, RNE)."""
    u = np.ascontiguousarray(a, dtype=np.float32).view(np.uint32)
    lsb = (u >> 12) & 1
    rounded = u + np.uint32(0x7FF) + lsb
    return (rounded & np.uint32(0xFFFFF000)).astype(np.uint32).view(np.float32)


def _build():
    nc = bacc.Bacc("TRN2", target_bir_lowering=False, debug=False,
                   enable_asserts=False)
    xd = nc.dram_tensor("x", [BPC, C, T], F32R, kind="ExternalInput").ap()
    wqT = nc.dram_tensor("wqT", [C, D], F32R, kind="ExternalInput").ap()
    wkT = nc.dram_tensor("wkT", [C, D], F32R, kind="ExternalInput").ap()
    wvT = nc.dram_tensor("wvT", [C, C], F32R, kind="ExternalInput").ap()
    bqd = nc.dram_tensor("bq", [D, 1], F32, kind="ExternalInput").ap()
    gbvd = nc.dram_tensor("gbv", [C, 1], F32, kind="ExternalInput").ap()
    onesd = nc.dram_tensor("ones", [128, 128], BF16, kind="ExternalInput").ap()
    outd = nc.dram_tensor("out", [BPC, C, T], F32, kind="ExternalOutput").ap()

    with tile.TileContext(nc) as tc:
        with tc.tile_pool(name="const", bufs=1) as constp, \
             tc.tile_pool(name="xp", bufs=2) as xp, \
             tc.tile_pool(name="vtp", bufs=1) as vtp, \
             tc.tile_pool(name="qkp", bufs=1) as qkp, \
             tc.tile_pool(name="etp", bufs=1) as etp, \
             tc.tile_pool(name="finp", bufs=1) as finp, \
             tc.tile_pool(name="ps", bufs=1, space="PSUM") as ps:

            # ---- x DMAs for sample 0 first (critical path), then weights ----
            x_sb_all = []
            for b in range(BPC):
                x_sb_all.append([xp.tile([128, T], F32R, name=f"x_{b}_{cc}",
                                         tag=f"x{cc}") for cc in range(CCH)])
            # quarter-granularity loads so dependent matmuls start early
            for q4 in range(4):
                qsl = slice(q4 * TW, (q4 + 1) * TW)
                for cc in range(CCH):
                    nc.sync.dma_start(
                        out=x_sb_all[0][cc][:, qsl],
                        in_=xd[0, cc * 128:(cc + 1) * 128, qsl])

            wv_sb, wq_sb, wk_sb, gbv_sb = [], [], [], []
            for cc in range(CCH):
                t_wv = constp.tile([128, C], F32R, name=f"wv{cc}")
                nc.sync.dma_start(out=t_wv, in_=wvT[cc * 128:(cc + 1) * 128, :])
                wv_sb.append(t_wv)
                t_wq = constp.tile([128, D], F32R, name=f"wq{cc}")
                nc.sync.dma_start(out=t_wq, in_=wqT[cc * 128:(cc + 1) * 128, :])
                wq_sb.append(t_wq)
                t_wk = constp.tile([128, D], F32R, name=f"wk{cc}")
                nc.sync.dma_start(out=t_wk, in_=wkT[cc * 128:(cc + 1) * 128, :])
                wk_sb.append(t_wk)
                t_gbv = constp.tile([128, 1], F32, name=f"gbv{cc}")
                nc.sync.dma_start(out=t_gbv, in_=gbvd[cc * 128:(cc + 1) * 128, :])
                gbv_sb.append(t_gbv)
            ones = constp.tile([128, 128], BF16)
            nc.sync.dma_start(out=ones, in_=onesd)
            bq_sb = constp.tile([D, 1], F32)
            nc.sync.dma_start(out=bq_sb, in_=bqd)

            for b in range(BPC):
                x_sb = x_sb_all[b]
                if b > 0:
                    for q4 in range(4):
                        qsl = slice(q4 * TW, (q4 + 1) * TW)
                        for cc in range(CCH):
                            nc.sync.dma_start(
                                out=x_sb[cc][:, qsl],
                                in_=xd[b, cc * 128:(cc + 1) * 128, qsl])

                # ---- v^T tiles: vt[sc][s=128, o=512] (bf16) ----
                vt_sb = []
                for sc in range(SCH):
                    vps = ps.tile([128, TW], F32, name=f"vps_{b}_{sc}",
                                  tag="stq", bufs=2)
                    for cc in range(CCH):
                        nc.tensor.matmul(
                            vps[:],
                            x_sb[cc][:, sc * 128:(sc + 1) * 128],
                            wv_sb[cc][:],
                            start=(cc == 0), stop=(cc == CCH - 1))
                    t_vt = vtp.tile([128, C], BF16, name=f"vt_{b}_{sc}",
                                    tag=f"vt{sc}")
                    nc.scalar.activation(out=t_vt[:], in_=vps[:], func=AF.Copy)
                    vt_sb.append(t_vt)

                # ---- q, k: [64, T] (f32r) ----
                q_sb = qkp.tile([D, T], F32R, name=f"q_{b}", tag="q")
                k_sb = qkp.tile([D, T], F32R, name=f"k_{b}", tag="k")
                for tc_i in range(TCH):
                    tsl = slice(tc_i * TW, (tc_i + 1) * TW)
                    qps = ps.tile([D, TW], F32, name=f"qps_{b}_{tc_i}",
                                  tag="stq", bufs=2)
                    for cc in range(CCH):
                        nc.tensor.matmul(qps[:], wq_sb[cc][:],
                                         x_sb[cc][:, tsl],
                                         start=(cc == 0), stop=(cc == CCH - 1))
                    nc.scalar.activation(out=q_sb[:, tsl], in_=qps[:],
                                         func=AF.Identity, bias=bq_sb[:],
                                         scale=1.0)
                    kps = ps.tile([D, TW], F32, name=f"kps_{b}_{tc_i}",
                                  tag="stq", bufs=2)
                    for cc in range(CCH):
                        nc.tensor.matmul(kps[:], wk_sb[cc][:],
                                         x_sb[cc][:, tsl],
                                         start=(cc == 0), stop=(cc == CCH - 1))
                    nc.scalar.activation(out=k_sb[:, tsl], in_=kps[:],
                                         func=AF.Copy)

                # ---- attention, one 512-wide t-chunk at a time ----
                for tc_i in range(TCH):
                    tsl = slice(tc_i * TW, (tc_i + 1) * TW)
                    den = ps.tile([128, TW], F32, name=f"den_{b}_{tc_i}",
                                  tag="den", bufs=2)
                    oacc = [ps.tile([128, TW], F32, name=f"o_{b}_{tc_i}_{cc}",
                                    tag=f"o{cc}") for cc in range(CCH)]
                    et = [etp.tile([128, TW], BF16, name=f"et_{b}_{tc_i}_{sc}",
                                   tag=f"et{sc}") for sc in range(SCH)]
                    stp = [None] * SCH

                    def emit_st(sc):
                        stp[sc] = ps.tile([128, TW], F32,
                                          name=f"st_{b}_{tc_i}_{sc}",
                                          tag="stq", bufs=2)
                        nc.tensor.matmul(
                            stp[sc][:], k_sb[:, sc * 128:(sc + 1) * 128],
                            q_sb[:, tsl], start=True, stop=True)
                        nc.scalar.activation(out=et[sc][:], in_=stp[sc][:],
                                             func=AF.Exp)

                    # software pipeline: S^T/exp of chunk sc+2 is emitted
                    # before the den/out matmuls of chunk sc, so the PE has
                    # independent work while ACT computes exp.
                    emit_st(0)
                    emit_st(1)
                    for sc in range(SCH):
                        nc.tensor.matmul(den[:], ones[:], et[sc][:],
                                         start=(sc == 0), stop=(sc == SCH - 1))
                        for cc in range(CCH):
                            nc.tensor.matmul(
                                oacc[cc][:],
                                vt_sb[sc][:, cc * 128:(cc + 1) * 128],
                                et[sc][:],
                                start=(sc == 0), stop=(sc == SCH - 1))
                        if sc + 2 < SCH:
                            emit_st(sc + 2)

                    # free the o/den PSUM banks quickly via ACT copies; the
                    # (slow) DVE reciprocal then runs off the critical path.
                    recip = finp.tile([128, TW], F32,
                                      name=f"rc_{b}_{tc_i}", tag="rc", bufs=2)
                    nc.vector.reciprocal(out=recip[:], in_=den[:])
                    o_sb = []
                    for cc in range(CCH):
                        t_o = finp.tile([128, TW], F32,
                                        name=f"ob_{b}_{tc_i}_{cc}",
                                        tag=f"ob{cc}", bufs=2)
                        nc.scalar.activation(out=t_o[:], in_=oacc[cc][:],
                                             func=AF.Copy)
                        o_sb.append(t_o)
                    for cc in range(CCH):
                        t_m = finp.tile([128, TW], F32,
                                        name=f"m_{b}_{tc_i}_{cc}", tag="m",
                                        bufs=2)
                        nc.vector.tensor_mul(t_m[:], o_sb[cc][:], recip[:])
                        t_f = finp.tile([128, TW], F32,
                                        name=f"f_{b}_{tc_i}_{cc}", tag="f",
                                        bufs=3)
                        nc.scalar.activation(out=t_f[:], in_=t_m[:],
                                             func=AF.Identity,
                                             bias=gbv_sb[cc][:], scale=1.0)
                        nc.vector.tensor_add(t_f[:], t_f[:],
                                             x_sb[cc][:, tsl].bitcast(F32))
                        nc.sync.dma_start(
                            out=outd[b, cc * 128:(cc + 1) * 128, tsl],
                            in_=t_f)
    nc.compile()
    return nc


def _get_nc():
    if "nc" not in _CACHE:
        _CACHE["nc"] = _build()
    return _CACHE["nc"]


def kernel(x, wq, bq, wk, bk, wv, bv, gamma):
    global LAST_EXEC_NS
    g = float(np.asarray(gamma).reshape(-1)[0])
    # fold gamma into the v path; bk cancels inside softmax
    wvT = _round_fp32r((g * np.asarray(wv, np.float32)).T)
    gbv = (g * np.asarray(bv, np.float32)).reshape(C, 1)
    wqT = _round_fp32r(np.asarray(wq, np.float32).T)
    wkT = _round_fp32r(np.asarray(wk, np.float32).T)
    bq2 = np.asarray(bq, np.float32).reshape(D, 1)
    import ml_dtypes
    ones = np.ones((128, 128), ml_dtypes.bfloat16)
    xr = _round_fp32r(np.asarray(x, np.float32))

    in_maps = []
    for core in range(NCORES):
        in_maps.append({
            "x": xr[core * BPC:(core + 1) * BPC],
            "wqT": wqT, "wkT": wkT, "wvT": wvT,
            "bq": bq2, "gbv": gbv, "ones": ones,
        })

    nc = _get_nc()
    res = run_bass_kernel_spmd(nc, in_maps, core_ids=list(range(NCORES)),
                               trace=PROFILE)
    LAST_EXEC_NS = res.exec_time_ns
    out = np.empty((B, C, T), np.float32)
    for core in range(NCORES):
        out[core * BPC:(core + 1) * BPC] = res.results[core]["out"]
    return out


# revision 3
# speedup vs baseline: 1.2488x; 1.1926x over previous
"""Trainium2 Bass kernel for an attention block (B=16, C=512, T=2048).

reference:
  q = wq@x + bq; k = wk@x + bk; v = wv@x + bv          (conv1x1 per sample)
  attn = softmax(q^T k over s); out = v @ attn^T
  result = gamma * out + x

Sharding: data-parallel over batch across 8 NeuronCores (2 samples/core),
weights replicated.

Device algorithm (per sample):
  - host folds gamma into wv, and gamma*bv + x into the residual xg
    (softmax rows sum to 1, so the v-bias is a per-channel constant);
    bk is dropped (a per-t constant in scores cancels in softmax over s).
  - scores path in float32r (fp32 w/ 11-bit mantissa, 2 PE cycles/row),
    v/softmax-weights path in bf16 (1 cycle/row); PSUM accum always fp32.
  - v^T[s,o] tiles via matmul(lhsT=x_bf16[c,s], rhs=(gamma*wv)^T[c,o])
  - q[d,t], k[d,s] via matmul(lhsT=wq^T/wk^T, rhs=x_f32r); bias only on q
  - per 512-wide t-chunk, for each 128-wide s-chunk (sw pipelined by 2,
    next chunk's first two S^T/exp pairs pre-emitted so the PE never
    starves across chunk boundaries -> HAM stays at K=8/8):
      S^T[s,t] = matmul(lhsT=k[:,s], rhs=q[:,t])      (f32r, K=64, N=512)
      E = exp(S^T)  (ACT, PSUM->SBUF bf16; no max-subtraction: |S|<~64)
      den += matmul(lhsT=ones128, rhs=E)              (bf16; sum over s,
                                                       broadcast on parts)
      out0[c,t] += matmul(lhsT=v^T[s,c], rhs=E)       (bf16; 4 c-chunks)
    then: out0 -> SBUF (ACT, frees PSUM), r = recip(den) (DVE),
          result = out0 * r + xg (DVE mul+add) -> DMA out
"""
import numpy as np
import ml_dtypes
import concourse.bass as bass
import concourse.bacc as bacc
import concourse.tile as tile
from concourse import mybir
from concourse.bass_utils import run_bass_kernel_spmd

F32 = mybir.dt.float32
F32R = mybir.dt.float32r
BF16 = mybir.dt.bfloat16
AF = mybir.ActivationFunctionType

B, C, T, D = 16, 512, 2048, 64
NCORES = 8
BPC = B // NCORES          # samples per core
CCH = C // 128             # 4 channel chunks
TW = 512                   # t tile width (matmul free dim)
TCH = T // TW              # 4 t chunks
SCH = T // 128             # 16 s chunks

PROFILE = False            # set True before calling kernel() to capture HW time
LAST_EXEC_NS = None
_CACHE = {}


def _round_fp32r(a: np.ndarray) -> np.ndarray:
    """Round fp32 to fp32r precision (11 explicit mantissa bits, RNE)."""
    u = np.ascontiguousarray(a, dtype=np.float32).view(np.uint32)
    lsb = (u >> 12) & 1
    rounded = u + np.uint32(0x7FF) + lsb
    return (rounded & np.uint32(0xFFFFF000)).astype(np.uint32).view(np.float32)


def _build():
    nc = bacc.Bacc("TRN2", target_bir_lowering=False, debug=False,
                   enable_asserts=False)
    xd = nc.dram_tensor("x", [BPC, C, T], F32R, kind="ExternalInput").ap()
    xbd = nc.dram_tensor("xb", [BPC, C, T], BF16, kind="ExternalInput").ap()
    xgd = nc.dram_tensor("xg", [BPC, C, T], F32, kind="ExternalInput").ap()
    wqT = nc.dram_tensor("wqT", [C, D], F32R, kind="ExternalInput").ap()
    wkT = nc.dram_tensor("wkT", [C, D], F32R, kind="ExternalInput").ap()
    wvT = nc.dram_tensor("wvT", [C, C], BF16, kind="ExternalInput").ap()
    bqd = nc.dram_tensor("bq", [D, 1], F32, kind="ExternalInput").ap()
    onesd = nc.dram_tensor("ones", [128, 128], BF16, kind="ExternalInput").ap()
    outd = nc.dram_tensor("out", [BPC, C, T], F32, kind="ExternalOutput").ap()

    with tile.TileContext(nc) as tc:
        with tc.tile_pool(name="const", bufs=1) as constp, \
             tc.tile_pool(name="xp", bufs=2) as xp, \
             tc.tile_pool(name="vtp", bufs=1) as vtp, \
             tc.tile_pool(name="qkp", bufs=1) as qkp, \
             tc.tile_pool(name="etp", bufs=1) as etp, \
             tc.tile_pool(name="finp", bufs=1) as finp, \
             tc.tile_pool(name="ps", bufs=1, space="PSUM") as ps:

            # ---- tiles for x (f32r for q/k, bf16 for v^T) ----
            x_sb_all, xb_sb_all = [], []
            for b in range(BPC):
                x_sb_all.append([xp.tile([128, T], F32R, name=f"x_{b}_{cc}",
                                         tag=f"x{cc}") for cc in range(CCH)])
                xb_sb_all.append([xp.tile([128, T], BF16, name=f"xb_{b}_{cc}",
                                          tag=f"xb{cc}") for cc in range(CCH)])

            def load_x(b, q4):
                qsl = slice(q4 * TW, (q4 + 1) * TW)
                for cc in range(CCH):
                    csl = slice(cc * 128, (cc + 1) * 128)
                    nc.sync.dma_start(out=x_sb_all[b][cc][:, qsl],
                                      in_=xd[b, csl, qsl])
                    nc.sync.dma_start(out=xb_sb_all[b][cc][:, qsl],
                                      in_=xbd[b, csl, qsl])

            # first quarter of sample 0, then weights, then the rest
            load_x(0, 0)
            wv_sb, wq_sb, wk_sb = [], [], []
            for cc in range(CCH):
                csl = slice(cc * 128, (cc + 1) * 128)
                t_wv = constp.tile([128, C], BF16, name=f"wv{cc}")
                nc.sync.dma_start(out=t_wv, in_=wvT[csl, :])
                wv_sb.append(t_wv)
                t_wq = constp.tile([128, D], F32R, name=f"wq{cc}")
                nc.sync.dma_start(out=t_wq, in_=wqT[csl, :])
                wq_sb.append(t_wq)
                t_wk = constp.tile([128, D], F32R, name=f"wk{cc}")
                nc.sync.dma_start(out=t_wk, in_=wkT[csl, :])
                wk_sb.append(t_wk)
            ones = constp.tile([128, 128], BF16)
            nc.sync.dma_start(out=ones, in_=onesd)
            bq_sb = constp.tile([D, 1], F32)
            nc.sync.dma_start(out=bq_sb, in_=bqd)
            for q4 in range(1, 4):
                load_x(0, q4)

            for b in range(BPC):
                x_sb, xb_sb = x_sb_all[b], xb_sb_all[b]
                if b > 0:
                    for q4 in range(4):
                        load_x(b, q4)

                # ---- v^T tiles (bf16): vt[sc][s=128, o=512] ----
                vt_sb = []
                for sc in range(SCH):
                    vps = ps.tile([128, TW], F32, name=f"vps_{b}_{sc}",
                                  tag="stq", bufs=2)
                    for cc in range(CCH):
                        nc.tensor.matmul(
                            vps[:],
                            xb_sb[cc][:, sc * 128:(sc + 1) * 128],
                            wv_sb[cc][:],
                            start=(cc == 0), stop=(cc == CCH - 1))
                    t_vt = vtp.tile([128, C], BF16, name=f"vt_{b}_{sc}",
                                    tag=f"vt{sc}")
                    nc.scalar.activation(out=t_vt[:], in_=vps[:], func=AF.Copy)
                    vt_sb.append(t_vt)

                # ---- q, k: [64, T] (f32r) ----
                q_sb = qkp.tile([D, T], F32R, name=f"q_{b}", tag="q")
                k_sb = qkp.tile([D, T], F32R, name=f"k_{b}", tag="k")
                for tc_i in range(TCH):
                    tsl = slice(tc_i * TW, (tc_i + 1) * TW)
                    qps = ps.tile([D, TW], F32, name=f"qps_{b}_{tc_i}",
                                  tag="stq", bufs=2)
                    for cc in range(CCH):
                        nc.tensor.matmul(qps[:], wq_sb[cc][:],
                                         x_sb[cc][:, tsl],
                                         start=(cc == 0), stop=(cc == CCH - 1))
                    nc.scalar.activation(out=q_sb[:, tsl], in_=qps[:],
                                         func=AF.Identity, bias=bq_sb[:],
                                         scale=1.0)
                    kps = ps.tile([D, TW], F32, name=f"kps_{b}_{tc_i}",
                                  tag="stq", bufs=2)
                    for cc in range(CCH):
                        nc.tensor.matmul(kps[:], wk_sb[cc][:],
                                         x_sb[cc][:, tsl],
                                         start=(cc == 0), stop=(cc == CCH - 1))
                    nc.scalar.activation(out=k_sb[:, tsl], in_=kps[:],
                                         func=AF.Copy)

                # ---- attention ----
                et = {}

                def emit_st(tc_i, sc):
                    tsl = slice(tc_i * TW, (tc_i + 1) * TW)
                    stp = ps.tile([128, TW], F32, name=f"st_{b}_{tc_i}_{sc}",
                                  tag="stq", bufs=2)
                    nc.tensor.matmul(
                        stp[:], k_sb[:, sc * 128:(sc + 1) * 128],
                        q_sb[:, tsl], start=True, stop=True)
                    t_et = etp.tile([128, TW], BF16,
                                    name=f"et_{b}_{tc_i}_{sc}", tag=f"et{sc}")
                    nc.scalar.activation(out=t_et[:], in_=stp[:], func=AF.Exp)
                    et[(tc_i, sc)] = t_et

                emit_st(0, 0)
                emit_st(0, 1)
                for tc_i in range(TCH):
                    tsl = slice(tc_i * TW, (tc_i + 1) * TW)
                    den = ps.tile([128, TW], F32, name=f"den_{b}_{tc_i}",
                                  tag="den", bufs=2)
                    oacc = [ps.tile([128, TW], F32, name=f"o_{b}_{tc_i}_{cc}",
                                    tag=f"o{cc}") for cc in range(CCH)]
                    # residual (+ gamma*bv) prefetch for this chunk
                    xg_sb = []
                    for cc in range(CCH):
                        t_xg = finp.tile([128, TW], F32,
                                         name=f"xg_{b}_{tc_i}_{cc}", tag="xg",
                                         bufs=6)
                        nc.sync.dma_start(
                            out=t_xg,
                            in_=xgd[b, cc * 128:(cc + 1) * 128, tsl])
                        xg_sb.append(t_xg)

                    for sc in range(SCH):
                        e = et.pop((tc_i, sc))
                        nc.tensor.matmul(den[:], ones[:], e[:],
                                         start=(sc == 0), stop=(sc == SCH - 1))
                        for cc in range(CCH):
                            nc.tensor.matmul(
                                oacc[cc][:],
                                vt_sb[sc][:, cc * 128:(cc + 1) * 128],
                                e[:], start=(sc == 0), stop=(sc == SCH - 1))
                        # stay 2 S^T/exp pairs ahead, crossing into the next
                        # t-chunk near the end so ACT keeps the PE fed
                        if sc + 2 < SCH:
                            emit_st(tc_i, sc + 2)
                        elif tc_i + 1 < TCH:
                            emit_st(tc_i + 1, sc + 2 - SCH)

                    # free o/den PSUM banks via ACT copies; slow DVE recip
                    # runs off the PE critical path
                    recip = finp.tile([128, TW], F32,
                                      name=f"rc_{b}_{tc_i}", tag="rc", bufs=2)
                    nc.vector.reciprocal(out=recip[:], in_=den[:])
                    for cc in range(CCH):
                        t_o = finp.tile([128, TW], F32,
                                        name=f"ob_{b}_{tc_i}_{cc}",
                                        tag=f"ob{cc}", bufs=2)
                        nc.scalar.activation(out=t_o[:], in_=oacc[cc][:],
                                             func=AF.Copy)
                        t_f = finp.tile([128, TW], F32,
                                        name=f"f_{b}_{tc_i}_{cc}", tag="f",
                                        bufs=3)
                        nc.vector.tensor_mul(t_f[:], t_o[:], recip[:])
                        nc.vector.tensor_add(t_f[:], t_f[:], xg_sb[cc][:])
                        nc.sync.dma_start(
                            out=outd[b, cc * 128:(cc + 1) * 128, tsl],
                            in_=t_f)
    nc.compile()
    return nc


def _get_nc():
    if "nc" not in _CACHE:
        _CACHE["nc"] = _build()
    return _CACHE["nc"]


def kernel(x, wq, bq, wk, bk, wv, bv, gamma):
    global LAST_EXEC_NS
    g = float(np.asarray(gamma).reshape(-1)[0])
    x = np.asarray(x, np.float32)
    # fold gamma into the v path; bk cancels inside softmax; the v bias
    # contributes gamma*bv per channel (softmax rows sum to 1) -> fold it
    # plus the residual into xg
    wvT = (g * np.asarray(wv, np.float32)).T.astype(ml_dtypes.bfloat16)
    wqT = _round_fp32r(np.asarray(wq, np.float32).T)
    wkT = _round_fp32r(np.asarray(wk, np.float32).T)
    bq2 = np.asarray(bq, np.float32).reshape(D, 1)
    gbv = (g * np.asarray(bv, np.float32)).reshape(1, C, 1)
    xg = x + gbv
    ones = np.ones((128, 128), ml_dtypes.bfloat16)
    xr = _round_fp32r(x)
    xb = x.astype(ml_dtypes.bfloat16)

    in_maps = []
    for core in range(NCORES):
        sl = slice(core * BPC, (core + 1) * BPC)
        in_maps.append({
            "x": xr[sl], "xb": xb[sl], "xg": xg[sl],
            "wqT": wqT, "wkT": wkT, "wvT": wvT,
            "bq": bq2, "ones": ones,
        })

    nc = _get_nc()
    res = run_bass_kernel_spmd(nc, in_maps, core_ids=list(range(NCORES)),
                               trace=PROFILE)
    LAST_EXEC_NS = res.exec_time_ns
    out = np.empty((B, C, T), np.float32)
    for core in range(NCORES):
        out[core * BPC:(core + 1) * BPC] = res.results[core]["out"]
    return out


# revision 4
# speedup vs baseline: 1.3712x; 1.0980x over previous
"""Trainium2 Bass kernel for an attention block (B=16, C=512, T=2048).

reference:
  q = wq@x + bq; k = wk@x + bk; v = wv@x + bv          (conv1x1 per sample)
  attn = softmax(q^T k over s); out = v @ attn^T
  result = gamma * out + x

Sharding: data-parallel over batch across 8 NeuronCores (2 samples/core),
weights replicated.

Device algorithm (per sample):
  - host folds gamma into wv, and gamma*bv + x into the residual xg
    (softmax rows sum to 1, so the v-bias is a per-channel constant);
    bk is dropped (a per-t constant in scores cancels in softmax over s).
  - q/k/scores path in fp16 (1 PE cycle/row, 11-bit mantissa incl
    implicit); v/softmax-weights path in bf16 (range: exp(S) up to e^64);
    PSUM accumulation always fp32.
  - v^T[s,o] tiles via matmul(lhsT=x_fp16[c,s], rhs=(gamma*wv)^T[c,o])
  - q[d,t], k[d,s] via matmul(lhsT=wq^T/wk^T, rhs=x_fp16); bias only on q
  - per 512-wide t-chunk, for each 128-wide s-chunk (sw pipelined by 2,
    next chunk's first two S^T/exp pairs pre-emitted so the PE never
    starves across chunk boundaries -> HAM stays at K=8/8):
      S^T[s,t] = matmul(lhsT=k[:,s], rhs=q[:,t])      (fp16, K=64, N=512)
      E = exp(S^T)  (ACT, PSUM->SBUF bf16; no max-subtraction: |S|<~64)
      den += matmul(lhsT=ones128, rhs=E)              (bf16; sum over s,
                                                       broadcast on parts)
      out0[c,t] += matmul(lhsT=v^T[s,c], rhs=E)       (bf16; 4 c-chunks)
    then: out0 -> SBUF (ACT, frees PSUM), r = recip(den) (DVE),
          result = out0 * r + xg (DVE mul+add) -> DMA out
"""
import numpy as np
import ml_dtypes
import concourse.bass as bass
import concourse.bacc as bacc
import concourse.tile as tile
from concourse import mybir
from concourse.bass_utils import run_bass_kernel_spmd

F32 = mybir.dt.float32
FP16 = mybir.dt.float16
BF16 = mybir.dt.bfloat16
AF = mybir.ActivationFunctionType

B, C, T, D = 16, 512, 2048, 64
NCORES = 8
BPC = B // NCORES          # samples per core
CCH = C // 128             # 4 channel chunks
TW = 512                   # t tile width (matmul free dim)
TCH = T // TW              # 4 t chunks
SCH = T // 128             # 16 s chunks

PROFILE = False            # set True before calling kernel() to capture HW time
LAST_EXEC_NS = None
_CACHE = {}


def _round_fp32r(a: np.ndarray) -> np.ndarray:
    """Round fp32 to fp32r precision (11 explicit mantissa bits, RNE)."""
    u = np.ascontiguousarray(a, dtype=np.float32).view(np.uint32)
    lsb = (u >> 12) & 1
    rounded = u + np.uint32(0x7FF) + lsb
    return (rounded & np.uint32(0xFFFFF000)).astype(np.uint32).view(np.float32)


def _build():
    nc = bacc.Bacc("TRN2", target_bir_lowering=False, debug=False,
                   enable_asserts=False)
    xd = nc.dram_tensor("x", [BPC, C, T], FP16, kind="ExternalInput").ap()
    xgd = nc.dram_tensor("xg", [BPC, C, T], F32, kind="ExternalInput").ap()
    wqT = nc.dram_tensor("wqT", [C, D], FP16, kind="ExternalInput").ap()
    wkT = nc.dram_tensor("wkT", [C, D], FP16, kind="ExternalInput").ap()
    wvT = nc.dram_tensor("wvT", [C, C], FP16, kind="ExternalInput").ap()
    bqd = nc.dram_tensor("bq", [D, 1], F32, kind="ExternalInput").ap()
    onesd = nc.dram_tensor("ones", [128, 128], BF16, kind="ExternalInput").ap()
    outd = nc.dram_tensor("out", [BPC, C, T], F32, kind="ExternalOutput").ap()

    with tile.TileContext(nc) as tc:
        with tc.tile_pool(name="const", bufs=1) as constp, \
             tc.tile_pool(name="xp", bufs=2) as xp, \
             tc.tile_pool(name="vtp", bufs=1) as vtp, \
             tc.tile_pool(name="qkp", bufs=1) as qkp, \
             tc.tile_pool(name="etp", bufs=1) as etp, \
             tc.tile_pool(name="finp", bufs=1) as finp, \
             tc.tile_pool(name="ps", bufs=1, space="PSUM") as ps:

            # ---- tiles for x (fp16, feeds q/k and v^T matmuls) ----
            x_sb_all = []
            for b in range(BPC):
                x_sb_all.append([xp.tile([128, T], FP16, name=f"x_{b}_{cc}",
                                         tag=f"x{cc}") for cc in range(CCH)])

            def load_x(b, q4):
                qsl = slice(q4 * TW, (q4 + 1) * TW)
                for cc in range(CCH):
                    csl = slice(cc * 128, (cc + 1) * 128)
                    nc.sync.dma_start(out=x_sb_all[b][cc][:, qsl],
                                      in_=xd[b, csl, qsl])

            # first quarter of sample 0, then weights, then the rest
            load_x(0, 0)
            wv_sb, wq_sb, wk_sb = [], [], []
            for cc in range(CCH):
                csl = slice(cc * 128, (cc + 1) * 128)
                t_wv = constp.tile([128, C], FP16, name=f"wv{cc}")
                nc.sync.dma_start(out=t_wv, in_=wvT[csl, :])
                wv_sb.append(t_wv)
                t_wq = constp.tile([128, D], FP16, name=f"wq{cc}")
                nc.sync.dma_start(out=t_wq, in_=wqT[csl, :])
                wq_sb.append(t_wq)
                t_wk = constp.tile([128, D], FP16, name=f"wk{cc}")
                nc.sync.dma_start(out=t_wk, in_=wkT[csl, :])
                wk_sb.append(t_wk)
            ones = constp.tile([128, 128], BF16)
            nc.sync.dma_start(out=ones, in_=onesd)
            bq_sb = constp.tile([D, 1], F32)
            nc.sync.dma_start(out=bq_sb, in_=bqd)
            for q4 in range(1, 4):
                load_x(0, q4)

            for b in range(BPC):
                x_sb = x_sb_all[b]
                if b > 0:
                    for q4 in range(4):
                        load_x(b, q4)

                # ---- v^T tiles (bf16): vt[sc][s=128, o=512] ----
                vt_sb = []
                for sc in range(SCH):
                    vps = ps.tile([128, TW], F32, name=f"vps_{b}_{sc}",
                                  tag="stq", bufs=2)
                    for cc in range(CCH):
                        nc.tensor.matmul(
                            vps[:],
                            x_sb[cc][:, sc * 128:(sc + 1) * 128],
                            wv_sb[cc][:],
                            start=(cc == 0), stop=(cc == CCH - 1))
                    t_vt = vtp.tile([128, C], BF16, name=f"vt_{b}_{sc}",
                                    tag=f"vt{sc}")
                    nc.scalar.activation(out=t_vt[:], in_=vps[:], func=AF.Copy)
                    vt_sb.append(t_vt)

                # ---- q, k: [64, T] (f32r) ----
                q_sb = qkp.tile([D, T], FP16, name=f"q_{b}", tag="q")
                k_sb = qkp.tile([D, T], FP16, name=f"k_{b}", tag="k")
                for tc_i in range(TCH):
                    tsl = slice(tc_i * TW, (tc_i + 1) * TW)
                    qps = ps.tile([D, TW], F32, name=f"qps_{b}_{tc_i}",
                                  tag="stq", bufs=2)
                    for cc in range(CCH):
                        nc.tensor.matmul(qps[:], wq_sb[cc][:],
                                         x_sb[cc][:, tsl],
                                         start=(cc == 0), stop=(cc == CCH - 1))
                    nc.scalar.activation(out=q_sb[:, tsl], in_=qps[:],
                                         func=AF.Identity, bias=bq_sb[:],
                                         scale=1.0)
                    kps = ps.tile([D, TW], F32, name=f"kps_{b}_{tc_i}",
                                  tag="stq", bufs=2)
                    for cc in range(CCH):
                        nc.tensor.matmul(kps[:], wk_sb[cc][:],
                                         x_sb[cc][:, tsl],
                                         start=(cc == 0), stop=(cc == CCH - 1))
                    nc.scalar.activation(out=k_sb[:, tsl], in_=kps[:],
                                         func=AF.Copy)

                # ---- attention ----
                et = {}

                def emit_st(tc_i, sc):
                    tsl = slice(tc_i * TW, (tc_i + 1) * TW)
                    stp = ps.tile([128, TW], F32, name=f"st_{b}_{tc_i}_{sc}",
                                  tag="stq", bufs=2)
                    nc.tensor.matmul(
                        stp[:], k_sb[:, sc * 128:(sc + 1) * 128],
                        q_sb[:, tsl], start=True, stop=True)
                    t_et = etp.tile([128, TW], BF16,
                                    name=f"et_{b}_{tc_i}_{sc}", tag=f"et{sc}")
                    nc.scalar.activation(out=t_et[:], in_=stp[:], func=AF.Exp)
                    et[(tc_i, sc)] = t_et

                emit_st(0, 0)
                emit_st(0, 1)
                for tc_i in range(TCH):
                    tsl = slice(tc_i * TW, (tc_i + 1) * TW)
                    den = ps.tile([128, TW], F32, name=f"den_{b}_{tc_i}",
                                  tag="den", bufs=2)
                    oacc = [ps.tile([128, TW], F32, name=f"o_{b}_{tc_i}_{cc}",
                                    tag=f"o{cc}") for cc in range(CCH)]
                    # residual (+ gamma*bv) prefetch for this chunk
                    xg_sb = []
                    for cc in range(CCH):
                        t_xg = finp.tile([128, TW], F32,
                                         name=f"xg_{b}_{tc_i}_{cc}", tag="xg",
                                         bufs=6)
                        nc.sync.dma_start(
                            out=t_xg,
                            in_=xgd[b, cc * 128:(cc + 1) * 128, tsl])
                        xg_sb.append(t_xg)

                    for sc in range(SCH):
                        e = et.pop((tc_i, sc))
                        nc.tensor.matmul(den[:], ones[:], e[:],
                                         start=(sc == 0), stop=(sc == SCH - 1))
                        for cc in range(CCH):
                            nc.tensor.matmul(
                                oacc[cc][:],
                                vt_sb[sc][:, cc * 128:(cc + 1) * 128],
                                e[:], start=(sc == 0), stop=(sc == SCH - 1))
                        # stay 2 S^T/exp pairs ahead, crossing into the next
                        # t-chunk near the end so ACT keeps the PE fed
                        if sc + 2 < SCH:
                            emit_st(tc_i, sc + 2)
                        elif tc_i + 1 < TCH:
                            emit_st(tc_i + 1, sc + 2 - SCH)

                    # free o/den PSUM banks via ACT copies; slow DVE recip
                    # runs off the PE critical path
                    recip = finp.tile([128, TW], F32,
                                      name=f"rc_{b}_{tc_i}", tag="rc", bufs=2)
                    nc.vector.reciprocal(out=recip[:], in_=den[:])
                    for cc in range(CCH):
                        t_o = finp.tile([128, TW], F32,
                                        name=f"ob_{b}_{tc_i}_{cc}",
                                        tag=f"ob{cc}", bufs=2)
                        nc.scalar.activation(out=t_o[:], in_=oacc[cc][:],
                                             func=AF.Copy)
                        t_f = finp.tile([128, TW], F32,
                                        name=f"f_{b}_{tc_i}_{cc}", tag="f",
                                        bufs=3)
                        nc.vector.tensor_mul(t_f[:], t_o[:], recip[:])
                        nc.vector.tensor_add(t_f[:], t_f[:], xg_sb[cc][:])
                        nc.sync.dma_start(
                            out=outd[b, cc * 128:(cc + 1) * 128, tsl],
                            in_=t_f)
    nc.compile()
    return nc


def _get_nc():
    if "nc" not in _CACHE:
        _CACHE["nc"] = _build()
    return _CACHE["nc"]


def kernel(x, wq, bq, wk, bk, wv, bv, gamma):
    global LAST_EXEC_NS
    g = float(np.asarray(gamma).reshape(-1)[0])
    x = np.asarray(x, np.float32)
    # fold gamma into the v path; bk cancels inside softmax; the v bias
    # contributes gamma*bv per channel (softmax rows sum to 1) -> fold it
    # plus the residual into xg
    wvT = np.ascontiguousarray((g * np.asarray(wv, np.float32)).T).astype(np.float16)
    wqT = np.ascontiguousarray(np.asarray(wq, np.float32).T).astype(np.float16)
    wkT = np.ascontiguousarray(np.asarray(wk, np.float32).T).astype(np.float16)
    bq2 = np.asarray(bq, np.float32).reshape(D, 1)
    gbv = (g * np.asarray(bv, np.float32)).reshape(1, C, 1)
    xg = x + gbv
    ones = np.ones((128, 128), ml_dtypes.bfloat16)
    xh = x.astype(np.float16)

    in_maps = []
    for core in range(NCORES):
        sl = slice(core * BPC, (core + 1) * BPC)
        in_maps.append({
            "x": xh[sl], "xg": xg[sl],
            "wqT": wqT, "wkT": wkT, "wvT": wvT,
            "bq": bq2, "ones": ones,
        })

    nc = _get_nc()
    res = run_bass_kernel_spmd(nc, in_maps, core_ids=list(range(NCORES)),
                               trace=PROFILE)
    LAST_EXEC_NS = res.exec_time_ns
    out = np.empty((B, C, T), np.float32)
    for core in range(NCORES):
        out[core * BPC:(core + 1) * BPC] = res.results[core]["out"]
    return out


# revision 5
# speedup vs baseline: 1.3929x; 1.0158x over previous
"""Trainium2 Bass kernel for an attention block (B=16, C=512, T=2048).

reference:
  q = wq@x + bq; k = wk@x + bk; v = wv@x + bv          (conv1x1 per sample)
  attn = softmax(q^T k over s); out = v @ attn^T
  result = gamma * out + x

Sharding: data-parallel over batch across 8 NeuronCores (2 samples/core),
weights replicated.

Device algorithm (per sample):
  - host folds gamma into wv, and gamma*bv + x into the residual xg
    (softmax rows sum to 1, so the v-bias is a per-channel constant);
    bk is dropped (a per-t constant in scores cancels in softmax over s).
  - q/k/scores path in fp16 (1 PE cycle/row, 11-bit mantissa incl
    implicit); v/softmax-weights path in bf16 (range: exp(S) up to e^64);
    PSUM accumulation always fp32.
  - v^T[s,o] tiles via matmul(lhsT=x_fp16[c,s], rhs=(gamma*wv)^T[c,o])
  - q[d,t], k[d,s] via matmul(lhsT=wq^T/wk^T, rhs=x_fp16); bias only on q
  - per 512-wide t-chunk, for each 128-wide s-chunk (sw pipelined by 2,
    next chunk's first two S^T/exp pairs pre-emitted so the PE never
    starves across chunk boundaries -> HAM stays at K=8/8):
      S^T[s,t] = matmul(lhsT=k[:,s], rhs=q[:,t])      (fp16, K=64, N=512)
      E = exp(S^T)  (ACT, PSUM->SBUF bf16; no max-subtraction: |S|<~64)
      den += matmul(lhsT=ones128, rhs=E)              (bf16; sum over s,
                                                       broadcast on parts)
      out0[c,t] += matmul(lhsT=v^T[s,c], rhs=E)       (bf16; 4 c-chunks)
    then: out0 -> SBUF (ACT, frees PSUM), r = recip(den) (DVE),
          result = out0 * r + xg (DVE mul+add) -> DMA out
"""
import numpy as np
import ml_dtypes
import concourse.bass as bass
import concourse.bacc as bacc
import concourse.tile as tile
from concourse import mybir
from concourse.bass_utils import run_bass_kernel_spmd

F32 = mybir.dt.float32
FP16 = mybir.dt.float16
BF16 = mybir.dt.bfloat16
AF = mybir.ActivationFunctionType

B, C, T, D = 16, 512, 2048, 64
NCORES = 8
BPC = B // NCORES          # samples per core
CCH = C // 128             # 4 channel chunks
TW = 512                   # t tile width (matmul free dim)
TCH = T // TW              # 4 t chunks
SCH = T // 128             # 16 s chunks

PROFILE = False            # set True before calling kernel() to capture HW time
LAST_EXEC_NS = None
_CACHE = {}


def _round_fp32r(a: np.ndarray) -> np.ndarray:
    """Round fp32 to fp32r precision (11 explicit mantissa bits, RNE)."""
    u = np.ascontiguousarray(a, dtype=np.float32).view(np.uint32)
    lsb = (u >> 12) & 1
    rounded = u + np.uint32(0x7FF) + lsb
    return (rounded & np.uint32(0xFFFFF000)).astype(np.uint32).view(np.float32)


def _build():
    nc = bacc.Bacc("TRN2", target_bir_lowering=False, debug=False,
                   enable_asserts=False)
    xd = nc.dram_tensor("x", [BPC, C, T], FP16, kind="ExternalInput").ap()
    xgd = nc.dram_tensor("xg", [BPC, C, T], F32, kind="ExternalInput").ap()
    wqT = nc.dram_tensor("wqT", [C, D], FP16, kind="ExternalInput").ap()
    wkT = nc.dram_tensor("wkT", [C, D], FP16, kind="ExternalInput").ap()
    wvT = nc.dram_tensor("wvT", [C, C], FP16, kind="ExternalInput").ap()
    bqd = nc.dram_tensor("bq", [D, 1], F32, kind="ExternalInput").ap()
    onesd = nc.dram_tensor("ones", [128, 128], BF16, kind="ExternalInput").ap()
    outd = nc.dram_tensor("out", [BPC, C, T], F32, kind="ExternalOutput").ap()

    with tile.TileContext(nc) as tc:
        with tc.tile_pool(name="const", bufs=1) as constp, \
             tc.tile_pool(name="xp", bufs=2) as xp, \
             tc.tile_pool(name="vtp", bufs=1) as vtp, \
             tc.tile_pool(name="qkp", bufs=1) as qkp, \
             tc.tile_pool(name="etp", bufs=1) as etp, \
             tc.tile_pool(name="finp", bufs=1) as finp, \
             tc.tile_pool(name="ps", bufs=1, space="PSUM") as ps:

            # ---- tiles for x (fp16, feeds q/k and v^T matmuls) ----
            x_sb_all = []
            for b in range(BPC):
                x_sb_all.append([xp.tile([128, T], FP16, name=f"x_{b}_{cc}",
                                         tag=f"x{cc}") for cc in range(CCH)])

            def load_x(b, q4):
                qsl = slice(q4 * TW, (q4 + 1) * TW)
                for cc in range(CCH):
                    csl = slice(cc * 128, (cc + 1) * 128)
                    nc.sync.dma_start(out=x_sb_all[b][cc][:, qsl],
                                      in_=xd[b, csl, qsl])

            # weights first (small), then x
            wv_sb, wq_sb, wk_sb = [], [], []
            for cc in range(CCH):
                csl = slice(cc * 128, (cc + 1) * 128)
                t_wv = constp.tile([128, C], FP16, name=f"wv{cc}")
                nc.sync.dma_start(out=t_wv, in_=wvT[csl, :])
                wv_sb.append(t_wv)
                t_wq = constp.tile([128, D], FP16, name=f"wq{cc}")
                nc.sync.dma_start(out=t_wq, in_=wqT[csl, :])
                wq_sb.append(t_wq)
                t_wk = constp.tile([128, D], FP16, name=f"wk{cc}")
                nc.sync.dma_start(out=t_wk, in_=wkT[csl, :])
                wk_sb.append(t_wk)
            ones = constp.tile([128, 128], BF16)
            nc.sync.dma_start(out=ones, in_=onesd)
            bq_sb = constp.tile([D, 1], F32)
            nc.sync.dma_start(out=bq_sb, in_=bqd)
            for q4 in range(4):
                load_x(0, q4)

            for b in range(BPC):
                x_sb = x_sb_all[b]
                if b > 0:
                    for q4 in range(4):
                        load_x(b, q4)

                # ---- v^T tiles (bf16): vt[sc][s=128, o=512] ----
                vt_sb = []
                for sc in range(SCH):
                    vps = ps.tile([128, TW], F32, name=f"vps_{b}_{sc}",
                                  tag=f"o{sc % 2}")
                    for cc in range(CCH):
                        nc.tensor.matmul(
                            vps[:],
                            x_sb[cc][:, sc * 128:(sc + 1) * 128],
                            wv_sb[cc][:],
                            start=(cc == 0), stop=(cc == CCH - 1))
                    t_vt = vtp.tile([128, C], BF16, name=f"vt_{b}_{sc}",
                                    tag=f"vt{sc}")
                    nc.scalar.activation(out=t_vt[:], in_=vps[:], func=AF.Copy)
                    vt_sb.append(t_vt)

                # ---- q, k: [64, T] (f32r) ----
                q_sb = qkp.tile([D, T], FP16, name=f"q_{b}", tag="q")
                k_sb = qkp.tile([D, T], FP16, name=f"k_{b}", tag="k")
                for tc_i in range(TCH):
                    tsl = slice(tc_i * TW, (tc_i + 1) * TW)
                    qps = ps.tile([D, TW], F32, name=f"qps_{b}_{tc_i}",
                                  tag="o2")
                    for cc in range(CCH):
                        nc.tensor.matmul(qps[:], wq_sb[cc][:],
                                         x_sb[cc][:, tsl],
                                         start=(cc == 0), stop=(cc == CCH - 1))
                    nc.scalar.activation(out=q_sb[:, tsl], in_=qps[:],
                                         func=AF.Identity, bias=bq_sb[:],
                                         scale=1.0)
                    kps = ps.tile([D, TW], F32, name=f"kps_{b}_{tc_i}",
                                  tag="o3")
                    for cc in range(CCH):
                        nc.tensor.matmul(kps[:], wk_sb[cc][:],
                                         x_sb[cc][:, tsl],
                                         start=(cc == 0), stop=(cc == CCH - 1))
                    nc.scalar.activation(out=k_sb[:, tsl], in_=kps[:],
                                         func=AF.Copy)

                # ---- attention ----
                et = {}

                def emit_st2(tc_i, pr):
                    # two fp16 S^T matmuls back to back (one bf16<->fp16
                    # dtype switch per pair instead of per matmul), one
                    # 1024-wide exp
                    tsl = slice(tc_i * TW, (tc_i + 1) * TW)
                    stp = ps.tile([128, 2 * TW], F32,
                                  name=f"st_{b}_{tc_i}_{pr}", tag="stp")
                    for h in range(2):
                        sc = 2 * pr + h
                        nc.tensor.matmul(
                            stp[:, h * TW:(h + 1) * TW],
                            k_sb[:, sc * 128:(sc + 1) * 128],
                            q_sb[:, tsl], start=True, stop=True)
                    t_et = etp.tile([128, 2 * TW], BF16,
                                    name=f"et_{b}_{tc_i}_{pr}", tag=f"et{pr}")
                    nc.scalar.activation(out=t_et[:], in_=stp[:], func=AF.Exp)
                    et[(tc_i, pr)] = t_et

                emit_st2(0, 0)
                for tc_i in range(TCH):
                    tsl = slice(tc_i * TW, (tc_i + 1) * TW)
                    den = ps.tile([128, TW], F32, name=f"den_{b}_{tc_i}",
                                  tag="den", bufs=2)
                    oacc = [ps.tile([128, TW], F32, name=f"o_{b}_{tc_i}_{cc}",
                                    tag=f"o{cc}") for cc in range(CCH)]
                    # residual (+ gamma*bv) prefetch for this chunk
                    xg_sb = []
                    for cc in range(CCH):
                        t_xg = finp.tile([128, TW], F32,
                                         name=f"xg_{b}_{tc_i}_{cc}", tag="xg",
                                         bufs=6)
                        nc.sync.dma_start(
                            out=t_xg,
                            in_=xgd[b, cc * 128:(cc + 1) * 128, tsl])
                        xg_sb.append(t_xg)

                    NPR = SCH // 2
                    for pr in range(NPR):
                        # next pair's S^T/exp first so ACT keeps the PE fed
                        if pr + 1 < NPR:
                            emit_st2(tc_i, pr + 1)
                        elif tc_i + 1 < TCH:
                            emit_st2(tc_i + 1, 0)
                        e = et.pop((tc_i, pr))
                        for h in range(2):
                            sc = 2 * pr + h
                            esl = e[:, h * TW:(h + 1) * TW]
                            nc.tensor.matmul(den[:], ones[:], esl,
                                             start=(sc == 0),
                                             stop=(sc == SCH - 1))
                            for cc in range(CCH):
                                nc.tensor.matmul(
                                    oacc[cc][:],
                                    vt_sb[sc][:, cc * 128:(cc + 1) * 128],
                                    esl, start=(sc == 0),
                                    stop=(sc == SCH - 1))

                    # free o/den PSUM banks via ACT copies; slow DVE recip
                    # runs off the PE critical path
                    recip = finp.tile([128, TW], F32,
                                      name=f"rc_{b}_{tc_i}", tag="rc", bufs=2)
                    nc.vector.reciprocal(out=recip[:], in_=den[:])
                    last = (tc_i == TCH - 1)
                    for cc in range(CCH):
                        if last:
                            # tail: o-banks are not needed soon, skip the
                            # bank-freeing copy and read PSUM directly
                            o_src = oacc[cc][:]
                        else:
                            t_o = finp.tile([128, TW], F32,
                                            name=f"ob_{b}_{tc_i}_{cc}",
                                            tag=f"ob{cc}", bufs=2)
                            nc.scalar.activation(out=t_o[:], in_=oacc[cc][:],
                                                 func=AF.Copy)
                            o_src = t_o[:]
                        t_f = finp.tile([128, TW], F32,
                                        name=f"f_{b}_{tc_i}_{cc}", tag="f",
                                        bufs=3)
                        nc.vector.tensor_mul(t_f[:], o_src, recip[:])
                        nc.vector.tensor_add(t_f[:], t_f[:], xg_sb[cc][:])
                        nc.sync.dma_start(
                            out=outd[b, cc * 128:(cc + 1) * 128, tsl],
                            in_=t_f)
    nc.compile()
    return nc


def _get_nc():
    if "nc" not in _CACHE:
        _CACHE["nc"] = _build()
    return _CACHE["nc"]


def kernel(x, wq, bq, wk, bk, wv, bv, gamma):
    global LAST_EXEC_NS
    g = float(np.asarray(gamma).reshape(-1)[0])
    x = np.asarray(x, np.float32)
    # fold gamma into the v path; bk cancels inside softmax; the v bias
    # contributes gamma*bv per channel (softmax rows sum to 1) -> fold it
    # plus the residual into xg
    wvT = np.ascontiguousarray((g * np.asarray(wv, np.float32)).T).astype(np.float16)
    wqT = np.ascontiguousarray(np.asarray(wq, np.float32).T).astype(np.float16)
    wkT = np.ascontiguousarray(np.asarray(wk, np.float32).T).astype(np.float16)
    bq2 = np.asarray(bq, np.float32).reshape(D, 1)
    gbv = (g * np.asarray(bv, np.float32)).reshape(1, C, 1)
    xg = x + gbv
    ones = np.ones((128, 128), ml_dtypes.bfloat16)
    xh = x.astype(np.float16)

    in_maps = []
    for core in range(NCORES):
        sl = slice(core * BPC, (core + 1) * BPC)
        in_maps.append({
            "x": xh[sl], "xg": xg[sl],
            "wqT": wqT, "wkT": wkT, "wvT": wvT,
            "bq": bq2, "ones": ones,
        })

    nc = _get_nc()
    res = run_bass_kernel_spmd(nc, in_maps, core_ids=list(range(NCORES)),
                               trace=PROFILE)
    LAST_EXEC_NS = res.exec_time_ns
    out = np.empty((B, C, T), np.float32)
    for core in range(NCORES):
        out[core * BPC:(core + 1) * BPC] = res.results[core]["out"]
    return out


# revision 6
# speedup vs baseline: 1.4566x; 1.0458x over previous
"""Trainium2 Bass kernel for an attention block (B=16, C=512, T=2048).

reference:
  q = wq@x + bq; k = wk@x + bk; v = wv@x + bv          (conv1x1 per sample)
  attn = softmax(q^T k over s); out = v @ attn^T
  result = gamma * out + x

Sharding: data-parallel over batch across 8 NeuronCores (2 samples/core),
weights replicated.

Device algorithm (per sample):
  - host folds gamma into wv, and gamma*bv + x into the residual xg
    (softmax rows sum to 1, so the v-bias is a per-channel constant);
    bk is dropped (a per-t constant in scores cancels in softmax over s).
  - q/k/scores path in fp16 (1 PE cycle/row, 11-bit mantissa incl
    implicit); v/softmax-weights path in bf16 (range: exp(S) up to e^64);
    PSUM accumulation always fp32.
  - v^T[s,o] tiles via matmul(lhsT=x_fp16[c,s], rhs=(gamma*wv)^T[c,o])
  - q[d,t], k[d,s] via matmul(lhsT=wq^T/wk^T, rhs=x_fp16); bias only on q
  - per 512-wide t-chunk, for each 128-wide s-chunk (sw pipelined by 2,
    next chunk's first two S^T/exp pairs pre-emitted so the PE never
    starves across chunk boundaries -> HAM stays at K=8/8):
      S^T[s,t] = matmul(lhsT=k[:,s], rhs=q[:,t])      (fp16, K=64, N=512)
      E = exp(S^T)  (ACT, PSUM->SBUF bf16; no max-subtraction: |S|<~64)
      den += matmul(lhsT=ones128, rhs=E)              (bf16; sum over s,
                                                       broadcast on parts)
      out0[c,t] += matmul(lhsT=v^T[s,c], rhs=E)       (bf16; 4 c-chunks)
    then: out0 -> SBUF (ACT, frees PSUM), r = recip(den) (DVE),
          result = out0 * r + xg (DVE mul+add) -> DMA out
"""
import numpy as np
import ml_dtypes
import concourse.bass as bass
import concourse.bacc as bacc
import concourse.tile as tile
from concourse import mybir
from concourse.bass_utils import run_bass_kernel_spmd

F32 = mybir.dt.float32
FP16 = mybir.dt.float16
BF16 = mybir.dt.bfloat16
AF = mybir.ActivationFunctionType

B, C, T, D = 16, 512, 2048, 64
NCORES = 8
BPC = B // NCORES          # samples per core
CCH = C // 128             # 4 channel chunks
TW = 512                   # t tile width (matmul free dim)
TCH = T // TW              # 4 t chunks
SCH = T // 128             # 16 s chunks

PROFILE = False            # set True before calling kernel() to capture HW time
LAST_EXEC_NS = None
_CACHE = {}


def _round_fp32r(a: np.ndarray) -> np.ndarray:
    """Round fp32 to fp32r precision (11 explicit mantissa bits, RNE)."""
    u = np.ascontiguousarray(a, dtype=np.float32).view(np.uint32)
    lsb = (u >> 12) & 1
    rounded = u + np.uint32(0x7FF) + lsb
    return (rounded & np.uint32(0xFFFFF000)).astype(np.uint32).view(np.float32)


def _build():
    nc = bacc.Bacc("TRN2", target_bir_lowering=False, debug=False,
                   enable_asserts=False)
    xd = nc.dram_tensor("x", [BPC, C, T], FP16, kind="ExternalInput").ap()
    xgd = nc.dram_tensor("xg", [BPC, C, T], F32, kind="ExternalInput").ap()
    wqT = nc.dram_tensor("wqT", [C, D], FP16, kind="ExternalInput").ap()
    wkT = nc.dram_tensor("wkT", [C, D], FP16, kind="ExternalInput").ap()
    wvT = nc.dram_tensor("wvT", [C, C], FP16, kind="ExternalInput").ap()
    bqd = nc.dram_tensor("bq", [D, 1], F32, kind="ExternalInput").ap()
    onesd = nc.dram_tensor("ones", [128, 128], BF16, kind="ExternalInput").ap()
    outd = nc.dram_tensor("out", [BPC, C, T], F32, kind="ExternalOutput").ap()

    with tile.TileContext(nc) as tc:
        with tc.tile_pool(name="const", bufs=1) as constp, \
             tc.tile_pool(name="xp", bufs=2) as xp, \
             tc.tile_pool(name="vtp", bufs=1) as vtp, \
             tc.tile_pool(name="qkp", bufs=1) as qkp, \
             tc.tile_pool(name="etp", bufs=1) as etp, \
             tc.tile_pool(name="finp", bufs=1) as finp, \
             tc.tile_pool(name="ps", bufs=1, space="PSUM") as ps:

            # ---- tiles for x (fp16, feeds q/k and v^T matmuls) ----
            x_sb_all = []
            for b in range(BPC):
                x_sb_all.append([xp.tile([128, T], FP16, name=f"x_{b}_{cc}",
                                         tag=f"x{cc}") for cc in range(CCH)])

            def load_x(b, q4):
                qsl = slice(q4 * TW, (q4 + 1) * TW)
                for cc in range(CCH):
                    csl = slice(cc * 128, (cc + 1) * 128)
                    nc.sync.dma_start(out=x_sb_all[b][cc][:, qsl],
                                      in_=xd[b, csl, qsl])

            # first x quarter (vT/qk critical path), then weights, then rest
            load_x(0, 0)
            wv_sb, wq_sb, wk_sb = [], [], []
            for cc in range(CCH):
                csl = slice(cc * 128, (cc + 1) * 128)
                t_wv = constp.tile([128, C], FP16, name=f"wv{cc}")
                nc.sync.dma_start(out=t_wv, in_=wvT[csl, :])
                wv_sb.append(t_wv)
                t_wq = constp.tile([128, D], FP16, name=f"wq{cc}")
                nc.sync.dma_start(out=t_wq, in_=wqT[csl, :])
                wq_sb.append(t_wq)
                t_wk = constp.tile([128, D], FP16, name=f"wk{cc}")
                nc.sync.dma_start(out=t_wk, in_=wkT[csl, :])
                wk_sb.append(t_wk)
            ones = constp.tile([128, 128], BF16)
            nc.sync.dma_start(out=ones, in_=onesd)
            bq_sb = constp.tile([D, 1], F32)
            nc.sync.dma_start(out=bq_sb, in_=bqd)
            for q4 in range(1, 4):
                load_x(0, q4)

            for b in range(BPC):
                x_sb = x_sb_all[b]
                if b > 0:
                    for q4 in range(4):
                        load_x(b, q4)

                # ---- v^T tiles (bf16): vt[sc][s=128, o=512] ----
                vt_sb = []
                for sc in range(SCH):
                    vps = ps.tile([128, TW], F32, name=f"vps_{b}_{sc}",
                                  tag=f"o{sc % 2}")
                    for cc in range(CCH):
                        nc.tensor.matmul(
                            vps[:],
                            x_sb[cc][:, sc * 128:(sc + 1) * 128],
                            wv_sb[cc][:],
                            start=(cc == 0), stop=(cc == CCH - 1))
                    t_vt = vtp.tile([128, C], BF16, name=f"vt_{b}_{sc}",
                                    tag=f"vt{sc}")
                    nc.scalar.activation(out=t_vt[:], in_=vps[:], func=AF.Copy)
                    vt_sb.append(t_vt)

                # ---- q, k: [64, T] (f32r) ----
                q_sb = qkp.tile([D, T], FP16, name=f"q_{b}", tag="q")
                k_sb = qkp.tile([D, T], FP16, name=f"k_{b}", tag="k")
                for tc_i in range(TCH):
                    tsl = slice(tc_i * TW, (tc_i + 1) * TW)
                    qps = ps.tile([D, TW], F32, name=f"qps_{b}_{tc_i}",
                                  tag="o2")
                    for cc in range(CCH):
                        nc.tensor.matmul(qps[:], wq_sb[cc][:],
                                         x_sb[cc][:, tsl],
                                         start=(cc == 0), stop=(cc == CCH - 1))
                    nc.scalar.activation(out=q_sb[:, tsl], in_=qps[:],
                                         func=AF.Identity, bias=bq_sb[:],
                                         scale=1.0)
                    kps = ps.tile([D, TW], F32, name=f"kps_{b}_{tc_i}",
                                  tag="o3")
                    for cc in range(CCH):
                        nc.tensor.matmul(kps[:], wk_sb[cc][:],
                                         x_sb[cc][:, tsl],
                                         start=(cc == 0), stop=(cc == CCH - 1))
                    nc.scalar.activation(out=k_sb[:, tsl], in_=kps[:],
                                         func=AF.Copy)

                # ---- attention ----
                et = {}

                def emit_st2(tc_i, pr):
                    # two fp16 S^T matmuls back to back (one bf16<->fp16
                    # dtype switch per pair instead of per matmul), one
                    # 1024-wide exp
                    tsl = slice(tc_i * TW, (tc_i + 1) * TW)
                    stp = ps.tile([128, 2 * TW], F32,
                                  name=f"st_{b}_{tc_i}_{pr}", tag="stp")
                    for h in range(2):
                        sc = 2 * pr + h
                        nc.tensor.matmul(
                            stp[:, h * TW:(h + 1) * TW],
                            k_sb[:, sc * 128:(sc + 1) * 128],
                            q_sb[:, tsl], start=True, stop=True)
                    t_et = etp.tile([128, 2 * TW], BF16,
                                    name=f"et_{b}_{tc_i}_{pr}", tag=f"et{pr}")
                    for h in range(2):
                        hs = slice(h * TW, (h + 1) * TW)
                        nc.scalar.activation(out=t_et[:, hs], in_=stp[:, hs],
                                             func=AF.Exp)
                    et[(tc_i, pr)] = t_et

                emit_st2(0, 0)
                for tc_i in range(TCH):
                    tsl = slice(tc_i * TW, (tc_i + 1) * TW)
                    den = ps.tile([128, TW], F32, name=f"den_{b}_{tc_i}",
                                  tag="den", bufs=2)
                    oacc = [ps.tile([128, TW], F32, name=f"o_{b}_{tc_i}_{cc}",
                                    tag=f"o{cc}") for cc in range(CCH)]
                    # residual (+ gamma*bv) prefetch for this chunk
                    xg_sb = []
                    for cc in range(CCH):
                        t_xg = finp.tile([128, TW], F32,
                                         name=f"xg_{b}_{tc_i}_{cc}", tag="xg",
                                         bufs=6)
                        nc.sync.dma_start(
                            out=t_xg,
                            in_=xgd[b, cc * 128:(cc + 1) * 128, tsl])
                        xg_sb.append(t_xg)

                    NPR = SCH // 2
                    for pr in range(NPR):
                        # next pair's S^T/exp first so ACT keeps the PE fed
                        if pr + 1 < NPR:
                            emit_st2(tc_i, pr + 1)
                        elif tc_i + 1 < TCH:
                            emit_st2(tc_i + 1, 0)
                        e = et.pop((tc_i, pr))
                        for h in range(2):
                            sc = 2 * pr + h
                            esl = e[:, h * TW:(h + 1) * TW]
                            nc.tensor.matmul(den[:], ones[:], esl,
                                             start=(sc == 0),
                                             stop=(sc == SCH - 1))
                            for cc in range(CCH):
                                nc.tensor.matmul(
                                    oacc[cc][:],
                                    vt_sb[sc][:, cc * 128:(cc + 1) * 128],
                                    esl, start=(sc == 0),
                                    stop=(sc == SCH - 1))

                    # free o/den PSUM banks via ACT copies; slow DVE recip
                    # runs off the PE critical path
                    recip = finp.tile([128, TW], F32,
                                      name=f"rc_{b}_{tc_i}", tag="rc", bufs=2)
                    nc.vector.reciprocal(out=recip[:], in_=den[:])
                    last = (b == BPC - 1 and tc_i == TCH - 1)
                    for cc in range(CCH):
                        if last:
                            # tail: o-banks are not needed soon, skip the
                            # bank-freeing copy and read PSUM directly
                            o_src = oacc[cc][:]
                        else:
                            t_o = finp.tile([128, TW], F32,
                                            name=f"ob_{b}_{tc_i}_{cc}",
                                            tag=f"ob{cc}", bufs=2)
                            nc.scalar.activation(out=t_o[:], in_=oacc[cc][:],
                                                 func=AF.Copy)
                            o_src = t_o[:]
                        t_f = finp.tile([128, TW], F32,
                                        name=f"f_{b}_{tc_i}_{cc}", tag="f",
                                        bufs=3)
                        nc.vector.tensor_mul(t_f[:], o_src, recip[:])
                        nc.vector.tensor_add(t_f[:], t_f[:], xg_sb[cc][:])
                        nc.sync.dma_start(
                            out=outd[b, cc * 128:(cc + 1) * 128, tsl],
                            in_=t_f)
    nc.compile()
    return nc


def _get_nc():
    if "nc" not in _CACHE:
        _CACHE["nc"] = _build()
    return _CACHE["nc"]


def kernel(x, wq, bq, wk, bk, wv, bv, gamma):
    global LAST_EXEC_NS
    g = float(np.asarray(gamma).reshape(-1)[0])
    x = np.asarray(x, np.float32)
    # fold gamma into the v path; bk cancels inside softmax; the v bias
    # contributes gamma*bv per channel (softmax rows sum to 1) -> fold it
    # plus the residual into xg
    wvT = np.ascontiguousarray((g * np.asarray(wv, np.float32)).T).astype(np.float16)
    wqT = np.ascontiguousarray(np.asarray(wq, np.float32).T).astype(np.float16)
    wkT = np.ascontiguousarray(np.asarray(wk, np.float32).T).astype(np.float16)
    bq2 = np.asarray(bq, np.float32).reshape(D, 1)
    gbv = (g * np.asarray(bv, np.float32)).reshape(1, C, 1)
    xg = x + gbv
    ones = np.ones((128, 128), ml_dtypes.bfloat16)
    xh = x.astype(np.float16)

    in_maps = []
    for core in range(NCORES):
        sl = slice(core * BPC, (core + 1) * BPC)
        in_maps.append({
            "x": xh[sl], "xg": xg[sl],
            "wqT": wqT, "wkT": wkT, "wvT": wvT,
            "bq": bq2, "ones": ones,
        })

    nc = _get_nc()
    res = run_bass_kernel_spmd(nc, in_maps, core_ids=list(range(NCORES)),
                               trace=PROFILE)
    LAST_EXEC_NS = res.exec_time_ns
    out = np.empty((B, C, T), np.float32)
    for core in range(NCORES):
        out[core * BPC:(core + 1) * BPC] = res.results[core]["out"]
    return out
